# revision 34
# baseline (speedup 1.0000x reference)
"""Trainium2 Bass kernel for causal self-attention (nn_CausalSelfAttention).

Problem (hardcoded):
    x:     [1, 4096, 1024] f32
    w_qkv: [1024, 3072] f32, b_qkv: [3072] f32
    w_out: [1024, 1024] f32, b_out: [1024] f32
    16 heads, head_dim 64, causal softmax attention.

Sharding: tensor-parallel over heads. 8 cores x 2 heads each. Each core
computes QKV for its heads, T^2 causal attention, and a partial output
projection; host sums the 8 partial projections (the all-reduce) and adds
biases.

Math notes (exact simplifications):
  - b_k drops out: S[t,s] += q_t . b_k is constant per query row; softmax is
    shift-invariant along s.
  - b_v reduces to a host-side constant: O_h = sum_s a_s (v_s + b_v_h)
    = sum_s a_s v_s + b_v_h (attention weights sum to 1), so its contribution
    to the output is the constant row b_v @ w_out.
  - b_q is applied on-device as a per-partition bias when copying Q^T out of
    PSUM (free).
  - Per-token softmax denominators commute with the per-head output
    projection, so we normalize O per head right before the projection.

All matmuls run in float32r (TF32-like, 1 cycle/row for N>=256 — measured
~1.5e-4 rms error vs fp64, 16x better than bf16 at the same speed).

Device layout (per core, SPMD; all 8 cores run the same program):
  - x^T is precomputed host-side as f32 [1024, 4096] so all QKV matmuls can
    stream it directly (contraction dim on partitions).
  - Q^T, K^T: [128 (2 heads x 64 dim), T] float32r tiles; V': [T, 2x(64+1)]
    float32r with a ones column appended per head so the attention-value
    matmul also produces the softmax denominator in partition 64.
  - Scores are computed transposed (S^T [keys, queries]) so the softmax
    reduction over keys is the matmul contraction, never a partition-axis
    reduction, and exp(S^T) feeds the AV matmul directly with no transposes.
  - Causal masking: tk-chunks strictly above the diagonal are skipped; the
    single partial 128x128 block per diagonal chunk is masked by multiplying
    exp by an upper-triangular 0/1 mask (exp of the skipped columns is never
    computed: matmul N-ranges shrink on diagonal chunks).
"""

import os

import numpy as np
import ml_dtypes

T = 4096
E = 1024
NCORES = 8
D = 64  # head dim
TQ = 512  # query tile (8 tiles)
NJ = T // TQ

_CACHE = {}

# Results of the last SPMD run (exec_time_ns etc.), for the local test harness.
LAST_RESULTS = None


def _build():
    import concourse.bacc as bacc
    import concourse.tile as tile
    import concourse.mybir as mybir

    f32 = mybir.dt.float32
    f32r = mybir.dt.float32r
    EXP = mybir.ActivationFunctionType.Exp

    nc = bacc.Bacc("TRN2", target_bir_lowering=False, debug=False)

    xT = nc.dram_tensor("xT", [E, T], f32r, kind="ExternalInput").ap()
    # per-core slice of w_qkv: cols [q(128) | k(128) | v(128)] for this core's
    # two heads
    wqkv = nc.dram_tensor("wqkv", [E, 384], f32r, kind="ExternalInput").ap()
    bq = nc.dram_tensor("bq", [128], f32, kind="ExternalInput").ap()
    wo = nc.dram_tensor("wo", [128, E], f32r, kind="ExternalInput").ap()
    # [sel_h0(128) | sel_h1(128)] selector row for the denominator broadcast
    sel_dram = nc.dram_tensor("sel", [1, 256], f32r, kind="ExternalInput").ap()
    mask_dram = nc.dram_tensor("mask", [128, 128], f32r, kind="ExternalInput").ap()
    ident_dram = nc.dram_tensor("ident", [128, 128], f32r, kind="ExternalInput").ap()
    y = nc.dram_tensor("y", [T, E], f32, kind="ExternalOutput").ap()

    with tile.TileContext(nc) as tc:
        with (
            tc.tile_pool(name="consts", bufs=1) as consts,
            tc.tile_pool(name="w", bufs=8) as wpool,
            tc.tile_pool(name="xt", bufs=16) as xtp,
            tc.tile_pool(name="qt", bufs=NJ) as qtp,
            tc.tile_pool(name="kt", bufs=NJ) as ktp,
            tc.tile_pool(name="v", bufs=NJ) as vp,
            tc.tile_pool(name="vts", bufs=2) as vtsp,
            tc.tile_pool(name="expst", bufs=6) as exp_p,
            tc.tile_pool(name="otn", bufs=2) as otnp,
            tc.tile_pool(name="bb", bufs=2) as bbp,
            tc.tile_pool(name="rd", bufs=4) as rdp,
            tc.tile_pool(name="ysb", bufs=3) as ysp,
            tc.tile_pool(name="mm_ps", bufs=2, space="PSUM") as mmp,
            tc.tile_pool(name="st_ps", bufs=2, space="PSUM") as stp,
            tc.tile_pool(name="op_ps", bufs=2, space="PSUM") as opp,
        ):
            # constants
            mask = consts.tile([128, 128], f32r)  # 1 where tq >= tk else 0
            nc.sync.dma_start(mask[:], mask_dram[:])
            ident = consts.tile([128, 128], f32r)
            nc.sync.dma_start(ident[:], ident_dram[:])
            sel = consts.tile([1, 256], f32r)
            nc.sync.dma_start(sel[:], sel_dram[:])
            bq_sb = consts.tile([128, 1], f32)
            nc.sync.dma_start(bq_sb[:, 0], bq[:])
            w_sb = []
            for e in range(8):
                w = wpool.tile([128, 384], f32r)
                nc.sync.dma_start(w[:], wqkv[128 * e : 128 * (e + 1), :])
                w_sb.append(w)
            # wo is not needed until the first output projection — load it
            # after the QKV weights so the first matmuls start sooner
            wo_sb = consts.tile([128, E], f32r)
            nc.sync.dma_start(wo_sb[:], wo[:])

            def emit_outproj(otn, t0):
                # partial output projection for the tile whose normalized
                # O^T is `otn` (tokens [t0, t0+TQ))
                for c in range(4):
                    ys = ysp.tile([128, E], f32, tag="ys", name=f"ys_{t0}_{c}")
                    for half in range(2):
                        yp = mmp.tile([128, 512], f32, tag="mm", name=f"yp_{t0}_{c}_{half}")
                        nc.tensor.matmul(
                            yp[:],
                            otn[:, 128 * c : 128 * (c + 1)],
                            wo_sb[:, 512 * half : 512 * (half + 1)],
                            start=True, stop=True,
                        )
                        # split the PSUM->SBUF copies across DVE and ACT so
                        # the boundary-time DVE queue stays short (the
                        # reciprocal chain that releases the next tile's
                        # accumulators runs on DVE)
                        if half == 0:
                            nc.vector.tensor_copy(ys[:, 0:512], yp[:])
                        else:
                            nc.scalar.copy(ys[:, 512:1024], yp[:])
                    nc.sync.dma_start(
                        y[t0 + 128 * c : t0 + 128 * (c + 1), :], ys[:]
                    )

            def norm_chain(pend):
                # finish the pending tile's normalization: broadcast 1/denom
                # to the head partitions via K=1 matmuls, then scale O'
                ops, rd0, rd1, t0 = pend
                bps = stp.tile([128, TQ], f32, tag="st", name=f"bps_{t0}")
                nc.tensor.matmul(
                    bps[:], sel[0:1, 0:128], rd0[:], start=True, stop=False
                )
                nc.tensor.matmul(
                    bps[:], sel[0:1, 128:256], rd1[:], start=False, stop=True
                )
                bb = bbp.tile([128, TQ], f32, tag="bb", name=f"bb_{t0}")
                nc.vector.reciprocal_approx_fast(bb[:], bps[:])
                otn = otnp.tile([128, TQ], f32r, tag="otn", name=f"otn_{t0}")
                nc.vector.tensor_mul(otn[0:64, :], ops[0][0:64, :], bb[0:64, :])
                nc.vector.tensor_mul(otn[64:128, :], ops[1][0:64, :], bb[64:128, :])
                return otn, t0

            pending = None
            prev_otn = None
            kt_tiles = []
            v_tiles = []
            for j in range(NJ):
                t0 = TQ * j
                # ---- load x^T tiles for this token block ----
                xts = []
                for e in range(8):
                    xt = xtp.tile([128, TQ], f32r)
                    nc.sync.dma_start(
                        xt[:], xT[128 * e : 128 * (e + 1), t0 : t0 + TQ]
                    )
                    xts.append(xt)

                # ---- Q^T, K^T, V^T via w-stationary matmuls ----
                ps_q = mmp.tile([128, TQ], f32, tag="mm")
                for e in range(8):
                    nc.tensor.matmul(
                        ps_q[:], w_sb[e][:, 0:128], xts[e][:],
                        start=(e == 0), stop=(e == 7),
                    )
                qt = qtp.tile([128, TQ], f32r)
                # fold b_q in as a per-partition bias
                nc.vector.tensor_scalar_add(qt[:], ps_q[:], bq_sb[:, 0:1])

                ps_k = mmp.tile([128, TQ], f32, tag="mm")
                for e in range(8):
                    nc.tensor.matmul(
                        ps_k[:], w_sb[e][:, 128:256], xts[e][:],
                        start=(e == 0), stop=(e == 7),
                    )
                kt = ktp.tile([128, TQ], f32r)
                nc.vector.tensor_copy(kt[:], ps_k[:])
                kt_tiles.append(kt)

                ps_vt = mmp.tile([128, TQ], f32, tag="mm")
                for e in range(8):
                    nc.tensor.matmul(
                        ps_vt[:], w_sb[e][:, 256:384], xts[e][:],
                        start=(e == 0), stop=(e == 7),
                    )
                vts = vtsp.tile([128, TQ], f32r)
                nc.vector.tensor_copy(vts[:], ps_vt[:])

                # transpose V^T -> V [tokens, vfeat]; per 128-token chunk c the
                # layout is [V_h0(64) | 1 | V_h1(64) | 1]
                vt = vp.tile([128, 4 * 130], f32r)
                nc.vector.memset(
                    vt.rearrange("p (c w) -> p c w", w=130)[:, :, 64::65].bitcast(
                        f32
                    ),
                    1.0,
                )
                for c in range(4):
                    ps_tr = mmp.tile([128, 128], f32r, tag="mm")
                    nc.tensor.transpose(
                        ps_tr[:], vts[:, 128 * c : 128 * (c + 1)], ident[:]
                    )
                    nc.vector.tensor_copy(
                        vt[:, 130 * c : 130 * c + 64], ps_tr[:, 0:64]
                    )
                    nc.vector.tensor_copy(
                        vt[:, 130 * c + 65 : 130 * c + 129], ps_tr[:, 64:128]
                    )
                v_tiles.append(vt)

                # ---- causal attention for query tile j (both heads) ----
                op0 = opp.tile([65, TQ], f32, tag="op")
                op1 = opp.tile([65, TQ], f32, tag="op")
                ops = [op0, op1]
                nchunks = 4 * j + 4
                for g in range(nchunks):
                    jj, c = divmod(g, 4)
                    r = g - 4 * j  # >= 0 on the block-diagonal
                    col0 = 128 * r if r >= 0 else 0
                    # both heads' scores in one [128, 1024] PSUM tile (2 banks)
                    st = stp.tile([128, 2 * TQ], f32, tag="st")
                    for h in range(2):
                        nc.tensor.matmul(
                            st[:, TQ * h + col0 : TQ * h + TQ],
                            kt_tiles[jj][64 * h : 64 * h + 64, 128 * c : 128 * (c + 1)],
                            qt[64 * h : 64 * h + 64, col0:TQ],
                            start=True, stop=True,
                        )
                    ex = exp_p.tile([128, 2 * TQ], f32r, tag="ex")
                    st3 = st.rearrange("p (h n) -> p h n", h=2)
                    ex3 = ex.rearrange("p (h n) -> p h n", h=2)
                    nc.scalar.activation(
                        ex3[:, :, col0:TQ], st3[:, :, col0:TQ], EXP, scale=0.125
                    )
                    if r >= 0:
                        for h in range(2):
                            nc.vector.tensor_mul(
                                ex[:, TQ * h + col0 : TQ * h + col0 + 128],
                                ex[:, TQ * h + col0 : TQ * h + col0 + 128],
                                mask[:],
                            )
                    for h in range(2):
                        nc.tensor.matmul(
                            ops[h][:, col0:TQ],
                            v_tiles[jj][:, 130 * c + 65 * h : 130 * c + 65 * h + 65],
                            ex[:, TQ * h + col0 : TQ * h + TQ],
                            start=(g == 0), stop=(g == nchunks - 1),
                            skip_group_check=True,
                        )

                # grab the two denominator rows as soon as the AV
                # accumulation finishes; the rest of the normalization is
                # deferred into the next iteration (after its QKV matmuls)
                rd0 = rdp.tile([1, TQ], f32r, tag="rd")
                rd1 = rdp.tile([1, TQ], f32r, tag="rd")
                with nc.allow_low_precision(reason="f32r rounding of denom"):
                    nc.vector.tensor_copy(rd0[:], ops[0][64:65, :])
                    nc.scalar.copy(rd1[:], ops[1][64:65, :])
                # emit the previous tile's output projection here: its (ready)
                # matmuls keep the PE busy while the reciprocal chain runs
                if prev_otn is not None:
                    emit_outproj(*prev_otn)
                prev_otn = norm_chain((ops, rd0, rd1, t0))

            emit_outproj(*prev_otn)

    nc.compile()
    return nc


def _prep_inputs(x, w_qkv, b_qkv, w_out, b_out):
    x = np.asarray(x, dtype=np.float32).reshape(T, E)
    w_qkv = np.asarray(w_qkv, dtype=np.float32)
    b_qkv = np.asarray(b_qkv, dtype=np.float32)
    w_out = np.asarray(w_out, dtype=np.float32)
    b_out = np.asarray(b_out, dtype=np.float32)

    xT = np.ascontiguousarray(x.T)
    mask = np.triu(np.ones((128, 128), dtype=np.float32))
    ident = np.eye(128, dtype=np.float32)
    sel = np.zeros((1, 256), dtype=np.float32)
    sel[0, 0:64] = 1.0
    sel[0, 192:256] = 1.0

    in_maps = []
    for cidx in range(NCORES):
        lo, hi = 128 * cidx, 128 * (cidx + 1)
        wq = w_qkv[:, lo:hi]
        wk = w_qkv[:, E + lo : E + hi]
        wv = w_qkv[:, 2 * E + lo : 2 * E + hi]
        wqkv_c = np.ascontiguousarray(np.concatenate([wq, wk, wv], axis=1))
        in_maps.append(
            {
                "xT": xT,
                "wqkv": wqkv_c,
                "bq": np.ascontiguousarray(b_qkv[lo:hi]),
                "wo": np.ascontiguousarray(w_out[lo:hi, :]),
                "sel": sel,
                "mask": mask,
                "ident": ident,
            }
        )
    # host-side constant: b_out plus the exact b_v contribution
    b_v = b_qkv[2 * E : 3 * E]
    const_row = b_out + b_v @ w_out
    return in_maps, const_row


def kernel(x, w_qkv, b_qkv, w_out, b_out):
    global LAST_RESULTS
    from concourse.bass_utils import run_bass_kernel_spmd

    if "nc" not in _CACHE:
        _CACHE["nc"] = _build()
    nc = _CACHE["nc"]

    in_maps, const_row = _prep_inputs(x, w_qkv, b_qkv, w_out, b_out)
    res = run_bass_kernel_spmd(nc, in_maps, core_ids=list(range(NCORES)))
    LAST_RESULTS = res

    out = np.zeros((T, E), dtype=np.float32)
    for r in res.results:
        out += r["y"]
    out += const_row[None, :].astype(np.float32)
    return out.reshape(1, T, E)


# revision 37
# speedup vs baseline: 1.0648x; 1.0648x over previous
"""Trainium2 Bass kernel for causal self-attention (nn_CausalSelfAttention).

Problem (hardcoded):
    x:     [1, 4096, 1024] f32
    w_qkv: [1024, 3072] f32, b_qkv: [3072] f32
    w_out: [1024, 1024] f32, b_out: [1024] f32
    16 heads, head_dim 64, causal softmax attention.

Sharding: tensor-parallel over heads. 8 cores x 2 heads each. Each core
computes QKV for its heads, T^2 causal attention, and a partial output
projection; host sums the 8 partial projections (the all-reduce) and adds
biases.

Math notes (exact simplifications):
  - b_k drops out: S[t,s] += q_t . b_k is constant per query row; softmax is
    shift-invariant along s.
  - b_v reduces to a host-side constant: O_h = sum_s a_s (v_s + b_v_h)
    = sum_s a_s v_s + b_v_h (attention weights sum to 1), so its contribution
    to the output is the constant row b_v @ w_out.
  - b_q is applied on-device as a per-partition bias when copying Q^T out of
    PSUM (free).
  - Per-token softmax denominators commute with the per-head output
    projection, so we normalize O per head right before the projection.

All matmuls run in float32r (TF32-like, 1 cycle/row for N>=256 — measured
~1.5e-4 rms error vs fp64, 16x better than bf16 at the same speed).

Device layout (per core, SPMD; all 8 cores run the same program):
  - x^T is precomputed host-side as f32 [1024, 4096] so all QKV matmuls can
    stream it directly (contraction dim on partitions).
  - Q^T, K^T: [128 (2 heads x 64 dim), T] float32r tiles; V': [T, 2x(64+1)]
    float32r with a ones column appended per head so the attention-value
    matmul also produces the softmax denominator in partition 64.
  - Scores are computed transposed (S^T [keys, queries]) so the softmax
    reduction over keys is the matmul contraction, never a partition-axis
    reduction, and exp(S^T) feeds the AV matmul directly with no transposes.
  - Causal masking: tk-chunks strictly above the diagonal are skipped; the
    single partial 128x128 block per diagonal chunk is masked by multiplying
    exp by an upper-triangular 0/1 mask (exp of the skipped columns is never
    computed: matmul N-ranges shrink on diagonal chunks).
"""

import os

import numpy as np
import ml_dtypes

T = 4096
E = 1024
NCORES = 8
D = 64  # head dim
TQ = 512  # query tile (8 tiles)
NJ = T // TQ

_CACHE = {}

# Results of the last SPMD run (exec_time_ns etc.), for the local test harness.
LAST_RESULTS = None


def _build():
    import concourse.bacc as bacc
    import concourse.tile as tile
    import concourse.mybir as mybir

    f32 = mybir.dt.float32
    f32r = mybir.dt.float32r
    EXP = mybir.ActivationFunctionType.Exp

    nc = bacc.Bacc("TRN2", target_bir_lowering=False, debug=False)

    xT = nc.dram_tensor("xT", [E, T], f32r, kind="ExternalInput").ap()
    # per-core slice of w_qkv: cols [q(128) | k(128) | v(128)] for this core's
    # two heads
    wqkv = nc.dram_tensor("wqkv", [E, 384], f32r, kind="ExternalInput").ap()
    bq = nc.dram_tensor("bq", [128], f32, kind="ExternalInput").ap()
    wo = nc.dram_tensor("wo", [128, E], f32r, kind="ExternalInput").ap()
    # [sel_h0(128) | sel_h1(128)] selector row for the denominator broadcast
    sel_dram = nc.dram_tensor("sel", [1, 256], f32r, kind="ExternalInput").ap()
    mask_dram = nc.dram_tensor("mask", [128, 128], f32r, kind="ExternalInput").ap()
    ident_dram = nc.dram_tensor("ident", [128, 128], f32r, kind="ExternalInput").ap()
    y = nc.dram_tensor("y", [T, E], f32, kind="ExternalOutput").ap()

    with tile.TileContext(nc) as tc:
        with (
            tc.tile_pool(name="consts", bufs=1) as consts,
            tc.tile_pool(name="w", bufs=8) as wpool,
            tc.tile_pool(name="xt", bufs=16) as xtp,
            tc.tile_pool(name="qt", bufs=NJ) as qtp,
            tc.tile_pool(name="kt", bufs=NJ) as ktp,
            tc.tile_pool(name="v", bufs=NJ) as vp,
            tc.tile_pool(name="vts", bufs=2) as vtsp,
            tc.tile_pool(name="expst", bufs=6) as exp_p,
            tc.tile_pool(name="otn", bufs=2) as otnp,
            tc.tile_pool(name="bb", bufs=2) as bbp,
            tc.tile_pool(name="rd", bufs=4) as rdp,
            tc.tile_pool(name="ysb", bufs=3) as ysp,
            tc.tile_pool(name="mm_ps", bufs=2, space="PSUM") as mmp,
            tc.tile_pool(name="st_ps", bufs=2, space="PSUM") as stp,
            tc.tile_pool(name="op_ps", bufs=2, space="PSUM") as opp,
        ):
            # constants
            mask = consts.tile([128, 128], f32r)  # 1 where tq >= tk else 0
            nc.sync.dma_start(mask[:], mask_dram[:])
            ident = consts.tile([128, 128], f32r)
            nc.sync.dma_start(ident[:], ident_dram[:])
            sel = consts.tile([1, 256], f32r)
            nc.sync.dma_start(sel[:], sel_dram[:])
            bq_sb = consts.tile([128, 1], f32)
            nc.sync.dma_start(bq_sb[:, 0], bq[:])
            wo_sb = consts.tile([128, E], f32r)
            nc.sync.dma_start(wo_sb[:], wo[:])
            w_sb = []
            for e in range(8):
                w = wpool.tile([128, 384], f32r)
                nc.sync.dma_start(w[:], wqkv[128 * e : 128 * (e + 1), :])
                w_sb.append(w)

            def emit_outproj(otn, t0):
                # partial output projection for the tile whose normalized
                # O^T is `otn` (tokens [t0, t0+TQ))
                for c in range(4):
                    ys = ysp.tile([128, E], f32, tag="ys", name=f"ys_{t0}_{c}")
                    for half in range(2):
                        yp = mmp.tile([128, 512], f32, tag="mm", name=f"yp_{t0}_{c}_{half}")
                        nc.tensor.matmul(
                            yp[:],
                            otn[:, 128 * c : 128 * (c + 1)],
                            wo_sb[:, 512 * half : 512 * (half + 1)],
                            start=True, stop=True,
                        )
                        nc.vector.tensor_copy(
                            ys[:, 512 * half : 512 * (half + 1)], yp[:]
                        )
                    nc.sync.dma_start(
                        y[t0 + 128 * c : t0 + 128 * (c + 1), :], ys[:]
                    )

            def norm_chain(pend):
                # finish the pending tile's normalization: broadcast 1/denom
                # to the head partitions via K=1 matmuls, then scale O'
                ops, rd0, rd1, t0 = pend
                bps = stp.tile([128, TQ], f32, tag="st", name=f"bps_{t0}")
                nc.tensor.matmul(
                    bps[:], sel[0:1, 0:128], rd0[:], start=True, stop=False
                )
                nc.tensor.matmul(
                    bps[:], sel[0:1, 128:256], rd1[:], start=False, stop=True
                )
                bb = bbp.tile([128, TQ], f32, tag="bb", name=f"bb_{t0}")
                nc.vector.reciprocal_approx_fast(bb[:], bps[:])
                otn = otnp.tile([128, TQ], f32r, tag="otn", name=f"otn_{t0}")
                nc.vector.tensor_mul(otn[0:64, :], ops[0][0:64, :], bb[0:64, :])
                nc.vector.tensor_mul(otn[64:128, :], ops[1][0:64, :], bb[64:128, :])
                return otn, t0

            pending = None
            prev_otn = None
            kt_tiles = []
            v_tiles = []
            for j in range(NJ):
                t0 = TQ * j
                # ---- load x^T tiles for this token block ----
                xts = []
                for e in range(8):
                    xt = xtp.tile([128, TQ], f32r)
                    nc.sync.dma_start(
                        xt[:], xT[128 * e : 128 * (e + 1), t0 : t0 + TQ]
                    )
                    xts.append(xt)

                # ---- Q^T, K^T, V^T via w-stationary matmuls ----
                ps_q = mmp.tile([128, TQ], f32, tag="mm")
                for e in range(8):
                    nc.tensor.matmul(
                        ps_q[:], w_sb[e][:, 0:128], xts[e][:],
                        start=(e == 0), stop=(e == 7),
                    )
                qt = qtp.tile([128, TQ], f32r)
                # fold b_q in as a per-partition bias
                nc.vector.tensor_scalar_add(qt[:], ps_q[:], bq_sb[:, 0:1])

                ps_k = mmp.tile([128, TQ], f32, tag="mm")
                for e in range(8):
                    nc.tensor.matmul(
                        ps_k[:], w_sb[e][:, 128:256], xts[e][:],
                        start=(e == 0), stop=(e == 7),
                    )
                kt = ktp.tile([128, TQ], f32r)
                nc.vector.tensor_copy(kt[:], ps_k[:])
                kt_tiles.append(kt)

                ps_vt = mmp.tile([128, TQ], f32, tag="mm")
                for e in range(8):
                    nc.tensor.matmul(
                        ps_vt[:], w_sb[e][:, 256:384], xts[e][:],
                        start=(e == 0), stop=(e == 7),
                    )
                vts = vtsp.tile([128, TQ], f32r)
                nc.vector.tensor_copy(vts[:], ps_vt[:])

                # transpose V^T -> V [tokens, vfeat]; per 128-token chunk c the
                # layout is [V_h0(64) | 1 | V_h1(64) | 1]
                vt = vp.tile([128, 4 * 130], f32r)
                nc.vector.memset(
                    vt.rearrange("p (c w) -> p c w", w=130)[:, :, 64::65].bitcast(
                        f32
                    ),
                    1.0,
                )
                for c in range(4):
                    ps_tr = mmp.tile([128, 128], f32r, tag="mm")
                    nc.tensor.transpose(
                        ps_tr[:], vts[:, 128 * c : 128 * (c + 1)], ident[:]
                    )
                    nc.vector.tensor_copy(
                        vt[:, 130 * c : 130 * c + 64], ps_tr[:, 0:64]
                    )
                    nc.vector.tensor_copy(
                        vt[:, 130 * c + 65 : 130 * c + 129], ps_tr[:, 64:128]
                    )
                v_tiles.append(vt)

                # ---- causal attention for query tile j (both heads) ----
                op0 = opp.tile([65, TQ], f32, tag="op")
                op1 = opp.tile([65, TQ], f32, tag="op")
                ops = [op0, op1]
                nchunks = 4 * j + 4
                for g in range(nchunks):
                    jj, c = divmod(g, 4)
                    r = g - 4 * j  # >= 0 on the block-diagonal
                    col0 = 128 * r if r >= 0 else 0
                    # both heads' scores in one [128, 1024] PSUM tile (2 banks)
                    st = stp.tile([128, 2 * TQ], f32, tag="st")
                    for h in range(2):
                        nc.tensor.matmul(
                            st[:, TQ * h + col0 : TQ * h + TQ],
                            kt_tiles[jj][64 * h : 64 * h + 64, 128 * c : 128 * (c + 1)],
                            qt[64 * h : 64 * h + 64, col0:TQ],
                            start=True, stop=True,
                        )
                    ex = exp_p.tile([128, 2 * TQ], f32r, tag="ex")
                    st3 = st.rearrange("p (h n) -> p h n", h=2)
                    ex3 = ex.rearrange("p (h n) -> p h n", h=2)
                    nc.scalar.activation(
                        ex3[:, :, col0:TQ], st3[:, :, col0:TQ], EXP, scale=0.125
                    )
                    if r >= 0:
                        for h in range(2):
                            nc.vector.tensor_mul(
                                ex[:, TQ * h + col0 : TQ * h + col0 + 128],
                                ex[:, TQ * h + col0 : TQ * h + col0 + 128],
                                mask[:],
                            )
                    for h in range(2):
                        nc.tensor.matmul(
                            ops[h][:, col0:TQ],
                            v_tiles[jj][:, 130 * c + 65 * h : 130 * c + 65 * h + 65],
                            ex[:, TQ * h + col0 : TQ * h + TQ],
                            start=(g == 0), stop=(g == nchunks - 1),
                            skip_group_check=True,
                        )

                # grab the two denominator rows as soon as the AV
                # accumulation finishes; the rest of the normalization is
                # deferred into the next iteration (after its QKV matmuls)
                rd0 = rdp.tile([1, TQ], f32r, tag="rd")
                rd1 = rdp.tile([1, TQ], f32r, tag="rd")
                with nc.allow_low_precision(reason="f32r rounding of denom"):
                    nc.vector.tensor_copy(rd0[:], ops[0][64:65, :])
                    nc.vector.tensor_copy(rd1[:], ops[1][64:65, :])
                # emit the previous tile's output projection here: its (ready)
                # matmuls keep the PE busy while the reciprocal chain runs
                if prev_otn is not None:
                    emit_outproj(*prev_otn)
                prev_otn = norm_chain((ops, rd0, rd1, t0))

            emit_outproj(*prev_otn)

    nc.compile()
    return nc


def _prep_inputs(x, w_qkv, b_qkv, w_out, b_out):
    x = np.asarray(x, dtype=np.float32).reshape(T, E)
    w_qkv = np.asarray(w_qkv, dtype=np.float32)
    b_qkv = np.asarray(b_qkv, dtype=np.float32)
    w_out = np.asarray(w_out, dtype=np.float32)
    b_out = np.asarray(b_out, dtype=np.float32)

    xT = np.ascontiguousarray(x.T)
    mask = np.triu(np.ones((128, 128), dtype=np.float32))
    ident = np.eye(128, dtype=np.float32)
    sel = np.zeros((1, 256), dtype=np.float32)
    sel[0, 0:64] = 1.0
    sel[0, 192:256] = 1.0

    in_maps = []
    for cidx in range(NCORES):
        lo, hi = 128 * cidx, 128 * (cidx + 1)
        wq = w_qkv[:, lo:hi]
        wk = w_qkv[:, E + lo : E + hi]
        wv = w_qkv[:, 2 * E + lo : 2 * E + hi]
        wqkv_c = np.ascontiguousarray(np.concatenate([wq, wk, wv], axis=1))
        in_maps.append(
            {
                "xT": xT,
                "wqkv": wqkv_c,
                "bq": np.ascontiguousarray(b_qkv[lo:hi]),
                "wo": np.ascontiguousarray(w_out[lo:hi, :]),
                "sel": sel,
                "mask": mask,
                "ident": ident,
            }
        )
    # host-side constant: b_out plus the exact b_v contribution
    b_v = b_qkv[2 * E : 3 * E]
    const_row = b_out + b_v @ w_out
    return in_maps, const_row


def kernel(x, w_qkv, b_qkv, w_out, b_out):
    global LAST_RESULTS
    from concourse.bass_utils import run_bass_kernel_spmd

    if "nc" not in _CACHE:
        _CACHE["nc"] = _build()
    nc = _CACHE["nc"]

    in_maps, const_row = _prep_inputs(x, w_qkv, b_qkv, w_out, b_out)
    res = run_bass_kernel_spmd(nc, in_maps, core_ids=list(range(NCORES)))
    LAST_RESULTS = res

    out = np.zeros((T, E), dtype=np.float32)
    for r in res.results:
        out += r["y"]
    out += const_row[None, :].astype(np.float32)
    return out.reshape(1, T, E)
